# revision 14
# baseline (speedup 1.0000x reference)
"""Trainium2 Bass kernel for a per-token fake-quantized Linear:

    y = fake_quant(fake_quant(x) @ W.T + b)      (per-token int8 symmetric)

x: [4, 2048, 4096] f32, W: [4096, 4096] f32, b: [4096] f32.

Strategy (8 NeuronCores, pure data parallel over tokens - zero collectives):
  - 8192 tokens / 8 cores = 1024 tokens per core; W, b replicated.
  - Per-token quantized x values are integers in [-127, 127], EXACTLY
    representable in bf16, so the matmul runs on TensorE in bf16 (integer
    q as the moving operand, host-pre-packed W.T bf16 stationary) with f32
    PSUM accumulation. The only precision loss vs the f32 reference is W's
    bf16 rounding (~0.8% rel err after output requant; gate is 2e-2).
  - Rounding is exact round-to-nearest-even via +/-1.5*2^23 magic adds.
  - The bias is folded into the matmul as a K=1 rank-1 update
    (b_row^T @ rinv_row) since s_x * rinv_x == 1.
  - z^T = Wb @ q^T is computed in output-transposed layout (W stationary,
    read once per token-half mega-pass); per-token output stats use
    absmax(y) = s_x * absmax(z), computed after an xbar transpose back to
    natural layout via DRAM staging.
  - v2 scheduling (the perf-critical part vs the first working version):
      * pass-2 (transpose-back + requant + store) for token half 0 is
        emitted BETWEEN the two matmul mega-passes so it executes under
        half 1's matmuls; only half 1's pass-2 remains as a tail.
      * zt staging is ONE contiguous [O, T] DRAM buffer so each token
        tile's transpose-back is a single 1 MiB xbar read, not 8 chunks.
      * th=1 streams W entirely on the sync ring; the scalar ring carries
        pass-2A transposes + stores (all xbar transposes stay on the
        scalar ring - transpose+copy on the SAME ring serializes safely,
        and scalar-transpose ++ sync-copy is the baseline-proven combo).
      * qt-h1 strips are xbar-transposed in 4 per-og batches interleaved
        with th=0 og4..7's W-odd prefetch instead of one 40us batch.
      * th=0 evacuations run on VectorE (idle then), th=1 on ScalarE
        (VectorE busy with pass-2A then).
      * W prefetch lookahead raised to 6 blocks (1.5 o-groups).
"""

import sys

if "/opt/trn_rl_repo" not in sys.path:
    sys.path.insert(0, "/opt/trn_rl_repo")

from contextlib import ExitStack

import ml_dtypes
import numpy as np

import concourse.bass as bass
import concourse.mybir as mybir
import concourse.tile as tile
from concourse import bacc
from concourse.bass import ds
from concourse.bass_utils import run_bass_kernel_spmd
from concourse.masks import make_identity

N_CORES = 8
P = 128
T = 1024          # tokens per core
K = 4096          # in features
O = 4096          # out features
TT = T // P       # 8 token tiles
KT = K // P       # 32 k tiles
TH = T // 2       # token half (512) = matmul N
OG = 512          # outputs per o-group (4 o-tiles -> 4 PSUM banks in flight)
NOG = O // OG     # 8 o-groups
OT_PER_G = OG // P  # 4

Q_MAX = 127.0
EPS = 1e-5
MAGIC = 1.5 * 2**23  # f32 add/sub forces round-to-nearest-even to integer
INV_QMAX = float(np.float32(1.0) / np.float32(Q_MAX))

F32 = mybir.dt.float32
BF16 = mybir.dt.bfloat16


def build():
    nc = bacc.Bacc()
    x_ext = nc.declare_dram_parameter("x", [T, K], F32, isOutput=False)
    wt_ext = nc.declare_dram_parameter("wt", [K, O], BF16, isOutput=False)
    b_ext = nc.declare_dram_parameter("b", [O], F32, isOutput=False)
    out_ext = nc.declare_dram_parameter("out", [T, O], F32, isOutput=True)

    with tile.TileContext(nc) as tc, ExitStack() as ctx:
        dram = ctx.enter_context(tc.tile_pool(name="dram", bufs=1, space="DRAM"))
        singles = ctx.enter_context(tc.tile_pool(name="singles", bufs=1))
        xp = ctx.enter_context(tc.tile_pool(name="xp", bufs=3))
        qp = ctx.enter_context(tc.tile_pool(name="qp", bufs=3))
        qt_pool = ctx.enter_context(tc.tile_pool(name="qt", bufs=1))
        sxp = ctx.enter_context(tc.tile_pool(name="sxp", bufs=1))
        stat = ctx.enter_context(tc.tile_pool(name="stat", bufs=3))
        wp = ctx.enter_context(tc.tile_pool(name="wp", bufs=6))
        ztp = ctx.enter_context(tc.tile_pool(name="ztp", bufs=6))
        znp = ctx.enter_context(tc.tile_pool(name="znp", bufs=2))
        yp = ctx.enter_context(tc.tile_pool(name="yp", bufs=2))
        psum = ctx.enter_context(tc.tile_pool(name="psum", bufs=6, space="PSUM"))
        tpp = ctx.enter_context(tc.tile_pool(name="tpp", bufs=2, space="PSUM"))

        # one contiguous [O, T] staging buffer: a token tile's transpose-back
        # is a single [O, 128] -> [128, O] xbar read
        zt_dram = dram.tile([O, T], BF16, tag="zt_dram", name="zt_dram")
        q_dram_h = {1: dram.tile([TH, K], BF16, tag="q_dram1", name="q_dram1")}
        rinv_dram = dram.tile([TT, P], F32, tag="rinv_dram")

        identity = singles.tile([P, P], BF16, tag="identity")
        make_identity(nc, identity)

        # bias row in bf16 (partition 0), for the K=1 bias matmul
        b_row = singles.tile([1, O], BF16, tag="b_row")
        nc.gpsimd.dma_start(out=b_row, in_=b_ext[:])  # gpsimd DMA casts f32->bf16

        # q^T strips, one per (token-half, k-tile): [128k, 512t] bf16
        qt_tiles = [
            [qt_pool.tile([P, TH], BF16, tag=f"qt{h}_{k}", name=f"qt{h}_{k}")
             for k in range(KT)]
            for h in range(2)
        ]

        # ---- pass 1: per-token scales + integer quant + q^T transposes ----
        # x is loaded in two 1 MiB half-rows per token tile, split across
        # the two HWDGE rings, so the loads prefetch deeply and neither
        # ring is blocked by a data-dependent DMA.
        KH = K // 2
        sx_tiles = []
        for t in range(TT):
            xh = []
            for i in range(2):
                x_half = xp.tile([P, KH], F32, tag="x_half")
                # x-h0 (t<4) splits across both rings for the fastest ramp;
                # x-h1 rides sync only so the scalar ring is free for og0-2
                # W prefetch from ~25us (x on both rings for all tiles
                # head-of-line blocks W behind the x stream; x on sync only
                # halves x bandwidth and starves the whole head)
                eng = nc.scalar if (t < 4 and i == 1) else nc.sync
                eng.dma_start(
                    out=x_half, in_=x_ext[ds(t * P, P), ds(i * KH, KH)]
                )
                xh.append(x_half)
            amh = stat.tile([P, 2], F32, tag="am_x")
            for i in range(2):
                nc.vector.tensor_reduce(
                    out=amh[:, i:i + 1], in_=xh[i], axis=mybir.AxisListType.X,
                    op=mybir.AluOpType.max, apply_absolute_value=True,
                )
            am = stat.tile([P, 1], F32, tag="am_c")
            nc.vector.tensor_reduce(
                out=am, in_=amh, axis=mybir.AxisListType.X,
                op=mybir.AluOpType.max,
            )
            sx = sxp.tile([P, 1], F32, tag=f"sx{t}", name=f"sx{t}")
            # s = max(absmax, EPS) * (1/127)
            nc.vector.tensor_scalar(
                out=sx, in0=am, scalar1=EPS, scalar2=INV_QMAX,
                op0=mybir.AluOpType.max, op1=mybir.AluOpType.mult,
            )
            rinv = stat.tile([P, 1], F32, tag="rinv_x")
            nc.vector.reciprocal(out=rinv, in_=sx)
            nc.gpsimd.dma_start(out=rinv_dram[t, :], in_=rinv[:, 0:1])
            h, row = t // (TT // 2), (t % (TT // 2)) * P
            for i in range(2):
                # r = x * rinv + MAGIC  (in place, gpsimd), q = r - MAGIC -> bf16
                nc.gpsimd.tensor_scalar(
                    out=xh[i], in0=xh[i], scalar1=rinv, scalar2=MAGIC,
                    op0=mybir.AluOpType.mult, op1=mybir.AluOpType.add,
                )
                q_half = qp.tile([P, KH], BF16, tag="q_half")
                nc.vector.tensor_scalar(
                    out=q_half, in0=xh[i], scalar1=MAGIC,
                    scalar2=None, op0=mybir.AluOpType.subtract,
                )
                if h == 0:
                    # first token half: PE-transpose q into the q^T strips
                    # (PE is idle during pass 1; its program order guarantees
                    # these run before the first matmuls that consume them)
                    for j in range(KT // 2):
                        k = i * (KT // 2) + j
                        tp = tpp.tile([P, P], BF16, tag="tp")
                        nc.tensor.transpose(
                            tp, q_half[:, ds(j * P, P)], identity
                        )
                        nc.scalar.copy(
                            out=qt_tiles[0][k][:, ds(row, P)], in_=tp
                        )
                else:
                    # second half: DRAM-staged xbar transposes (no hurry)
                    nc.gpsimd.dma_start(
                        out=q_dram_h[1][ds(row, P), ds(i * KH, KH)], in_=q_half
                    )
            sx_tiles.append(sx)

        # rinv as a bf16 row vector [1, T] (rhs of the K=1 bias matmul)
        rinv_row = singles.tile([1, T], BF16, tag="rinv_row")
        nc.gpsimd.dma_start(out=rinv_row, in_=rinv_dram[:, :])

        KB = 8                       # k-subtiles per W block
        NKB = KT // KB               # 4 blocks per o-group

        def matmul_og(th, og, w_engine_of, evac_eng):
            ps = [
                psum.tile([P, TH], F32, tag="ps", name=f"ps_{th}_{og}_{i}")
                for i in range(OT_PER_G)
            ]
            for kb in range(NKB):
                w_tile = wp.tile([P, KB, OG], BF16, tag="w_tile")
                w_engine_of(kb).dma_start(
                    out=w_tile,
                    in_=wt_ext[
                        ds(kb * KB * P, KB * P), ds(og * OG, OG)
                    ].rearrange("(s p) o -> p s o", p=P),
                )
                for s in range(KB):
                    k = kb * KB + s
                    for ot in range(OT_PER_G):
                        nc.tensor.matmul(
                            ps[ot],
                            w_tile[:, s, ds(ot * P, P)],
                            qt_tiles[th][k],
                            start=(k == 0),
                            stop=False,
                        )
            # bias: psum += b_chunk^T @ rinv_row   (K=1 matmul)
            for ot in range(OT_PER_G):
                o0 = og * OG + ot * P
                nc.tensor.matmul(
                    ps[ot],
                    b_row[0:1, ds(o0, P)],
                    rinv_row[0:1, ds(th * TH, TH)],
                    start=False,
                    stop=True,
                )
            for ot in range(OT_PER_G):
                zt_sb = ztp.tile([P, TH], BF16, tag="zt_sb")
                if evac_eng is nc.vector:
                    evac_eng.tensor_copy(out=zt_sb, in_=ps[ot])
                else:
                    evac_eng.copy(out=zt_sb, in_=ps[ot])
                # evacs are data-dependent: on a FIFO HWDGE ring they
                # head-of-line block later W prefetch, so use SWDGE
                nc.gpsimd.dma_start(
                    out=zt_dram[ds(og * OG + ot * P, P), ds(th * TH, TH)],
                    in_=zt_sb,
                )

        def pass2_tile(t, store_eng):
            """Transpose token tile t back to natural layout, requant, store."""
            z_nat = znp.tile([P, O], BF16, tag="z_nat")
            # single 1 MiB xbar transposed read: [O, 128] -> [128, O]
            nc.scalar.dma_start_transpose(
                z_nat, zt_dram[:, ds(t * P, P)]
            )
            # per-token absmax of y comes from z: absmax(y) = s_x * absmax(z)
            OH = O // 2
            am = stat.tile([P, 1], F32, tag="am_z")
            nc.vector.tensor_reduce(
                out=am, in_=z_nat, axis=mybir.AxisListType.X,
                op=mybir.AluOpType.max, apply_absolute_value=True,
            )
            sy = stat.tile([P, 1], F32, tag="sy")
            # sy = (max(am * sx, EPS)) * (1/127)
            nc.vector.tensor_scalar(
                out=sy, in0=am, scalar1=sx_tiles[t], scalar2=EPS,
                op0=mybir.AluOpType.mult, op1=mybir.AluOpType.max,
            )
            nc.vector.tensor_scalar(
                out=sy, in0=sy, scalar1=INV_QMAX, scalar2=None,
                op0=mybir.AluOpType.mult,
            )
            rinvy = stat.tile([P, 1], F32, tag="rinv_y")
            nc.vector.reciprocal(out=rinvy, in_=sy)
            # f1 = s_x * rinv_y: ONE scalar-engine activation then does
            # r = z*f1 + MAGIC (bias already inside z)
            f1 = stat.tile([P, 1], F32, tag="f1")
            nc.vector.tensor_scalar(
                out=f1, in0=rinvy, scalar1=sx_tiles[t], scalar2=None,
                op0=mybir.AluOpType.mult,
            )
            for i in range(2):
                y_half = yp.tile([P, OH], F32, tag="y_half")
                # r = z * (sx*rinvy) + MAGIC  (scalar ACT, fused affine)
                nc.scalar.activation(
                    out=y_half, in_=z_nat[:, ds(i * OH, OH)],
                    func=mybir.ActivationFunctionType.Copy,
                    bias=MAGIC, scale=f1,
                )
                # y_q = (r - MAGIC) * s_y  (in place; vector ONLY - gpsimd
                # tensor_scalar with an AP in the scalar2 slot measured
                # 29.3us/op vs 1.1us on vector)
                nc.vector.tensor_scalar(
                    out=y_half, in0=y_half, scalar1=MAGIC, scalar2=sy,
                    op0=mybir.AluOpType.subtract, op1=mybir.AluOpType.mult,
                )
                store_eng.dma_start(
                    out=out_ext[ds(t * P, P), ds(i * OH, OH)], in_=y_half
                )

        # ---- matmul mega-pass th=0: while x drains on the sync ring,
        # og0-2's W rides the scalar ring alone; og3+ splits across both.
        # th=0 evacs go on ScalarE: VectorE's queue is full of t4-7 quant
        # work, and evacs queued behind it stall og2+ on psum slots.
        # qt-h1 strips go as ONE batch after og3: the scalar ring drains
        # them by ~100us, well before og4-7's W-odd is needed ----
        for og in range(NOG):
            if og < 3:
                w_engine_of = lambda kb: nc.scalar
            else:
                w_engine_of = lambda kb: nc.sync if kb % 2 == 0 else nc.scalar
            matmul_og(0, og, w_engine_of=w_engine_of, evac_eng=nc.scalar)
            if og == 3:
                for k in range(KT):
                    nc.scalar.dma_start_transpose(
                        qt_tiles[1][k], q_dram_h[1][:, ds(k * P, P)]
                    )

        # ---- pass-2A: token tiles 0..3 (executes under th=1 matmuls) ----
        for t in range(TT // 2):
            pass2_tile(t, store_eng=nc.scalar)

        # ---- matmul mega-pass th=1: W entirely on the sync ring; evacs on
        # VectorE (ScalarE carries pass-2A's ACTs + transposes then) ----
        for og in range(NOG):
            matmul_og(
                1, og,
                w_engine_of=lambda kb: nc.sync,
                evac_eng=nc.vector,
            )

        # ---- pass-2B tail: token tiles 4..7 ----
        for t in range(TT // 2, TT):
            pass2_tile(t, store_eng=nc.sync)

    nc.compile()
    return nc


_NC_CACHE = None


def _get_nc():
    global _NC_CACHE
    if _NC_CACHE is None:
        _NC_CACHE = build()
    return _NC_CACHE


def _run(x, W, b, trace=False):
    nc = _get_nc()
    x2d = np.ascontiguousarray(np.asarray(x, dtype=np.float32).reshape(-1, K))
    wt = np.ascontiguousarray(np.asarray(W, dtype=np.float32).T).astype(
        ml_dtypes.bfloat16
    )
    bf = np.ascontiguousarray(np.asarray(b, dtype=np.float32))
    in_maps = [
        {"x": np.ascontiguousarray(x2d[i * T:(i + 1) * T]), "wt": wt, "b": bf}
        for i in range(N_CORES)
    ]
    res = run_bass_kernel_spmd(nc, in_maps, list(range(N_CORES)), trace=trace)
    out = np.concatenate([res.results[i]["out"] for i in range(N_CORES)], axis=0)
    return out, res


def kernel(x, W, b):
    out, _ = _run(x, W, b, trace=False)
    return out.reshape(np.asarray(x).shape[:-1] + (O,)).astype(np.float32)


# revision 15
# speedup vs baseline: 1.0418x; 1.0418x over previous
"""Trainium2 Bass kernel for a per-token fake-quantized Linear:

    y = fake_quant(fake_quant(x) @ W.T + b)      (per-token int8 symmetric)

x: [4, 2048, 4096] f32, W: [4096, 4096] f32, b: [4096] f32.

Strategy (8 NeuronCores, pure data parallel over tokens - zero collectives):
  - 8192 tokens / 8 cores = 1024 tokens per core; W, b replicated.
  - Per-token quantized x values are integers in [-127, 127], EXACTLY
    representable in bf16, so the matmul runs on TensorE in bf16 (integer
    q as the moving operand, host-pre-packed W.T bf16 stationary) with f32
    PSUM accumulation. The only precision loss vs the f32 reference is W's
    bf16 rounding (~0.8% rel err after output requant; gate is 2e-2).
  - Rounding is exact round-to-nearest-even via +/-1.5*2^23 magic adds.
  - The bias is folded into the matmul as a K=1 rank-1 update
    (b_chunk^T @ rinv_row_h) since s_x * rinv_x == 1.
  - z^T = Wb @ q^T is computed in output-transposed layout (W stationary,
    read once per token-half mega-pass); per-token output stats use
    absmax(y) = s_x * absmax(z) after an xbar transpose back to natural
    layout via one contiguous [O, T] DRAM staging buffer (one 1 MiB
    transposed read per token tile).
  - Scheduling (the perf-critical part; each point trace-verified):
      * x halves and W blocks SHARE one 10-slot SBUF pool (equal 8 KiB
        per-partition slots). A separate W pool paces W-issue off matmul
        completion via slot-WAR semaphores and the pipeline never fills
        (8-35us PE gaps at og0-3); sharing lets early W reuse x slots
        freed at quant speed.
      * Quant is emitted in two batches: tiles 0-3 before the og loop,
        tiles 4-7 between og3 and og4. x-h1 then loads in og0-3's shadow
        and og0-3's zt evac-writes are not queued behind t4-7's
        quant work on gpsimd.
      * rinv_row is per-half so og0's bias matmul never waits on t4-7.
      * th=0 evacs on ScalarE, th=1 on VectorE (whichever queue is free).
      * qt-h1 xbar strips: one batch after og5 (q_dram ready ~115us),
        og6-7's W moves fully to sync so the batch can't delay it.
      * pass-2 for tokens 0-511 is emitted after th=1's og1 so its
        DMA-lane completions can't stall th1-og0/og1's W waits; it
        executes under th1's matmuls. Only tokens 512-1023's pass-2
        remains as the tail.
      * All xbar transposes stay on the scalar ring (transpose on scalar
        concurrent with copy on sync is the proven-safe combo).
"""

import sys

if "/opt/trn_rl_repo" not in sys.path:
    sys.path.insert(0, "/opt/trn_rl_repo")

from contextlib import ExitStack

import ml_dtypes
import numpy as np

import concourse.bass as bass
import concourse.mybir as mybir
import concourse.tile as tile
from concourse import bacc
from concourse.bass import ds
from concourse.bass_utils import run_bass_kernel_spmd
from concourse.masks import make_identity

N_CORES = 8
P = 128
T = 1024          # tokens per core
K = 4096          # in features
O = 4096          # out features
TT = T // P       # 8 token tiles
KT = K // P       # 32 k tiles
TH = T // 2       # token half (512) = matmul N
OG = 512          # outputs per o-group (4 o-tiles -> 4 PSUM banks in flight)
NOG = O // OG     # 8 o-groups
OT_PER_G = OG // P  # 4

Q_MAX = 127.0
EPS = 1e-5
MAGIC = 1.5 * 2**23  # f32 add/sub forces round-to-nearest-even to integer
INV_QMAX = float(np.float32(1.0) / np.float32(Q_MAX))

F32 = mybir.dt.float32
BF16 = mybir.dt.bfloat16


def build():
    nc = bacc.Bacc()
    x_ext = nc.declare_dram_parameter("x", [T, K], F32, isOutput=False)
    wt_ext = nc.declare_dram_parameter("wt", [K, O], BF16, isOutput=False)
    b_ext = nc.declare_dram_parameter("b", [O], F32, isOutput=False)
    out_ext = nc.declare_dram_parameter("out", [T, O], F32, isOutput=True)

    with tile.TileContext(nc) as tc, ExitStack() as ctx:
        dram = ctx.enter_context(tc.tile_pool(name="dram", bufs=1, space="DRAM"))
        singles = ctx.enter_context(tc.tile_pool(name="singles", bufs=1))
        # x halves AND W blocks share this pool: both are 8 KiB/partition
        stream = ctx.enter_context(tc.tile_pool(name="stream", bufs=10))
        qp = ctx.enter_context(tc.tile_pool(name="qp", bufs=3))
        qt_pool = ctx.enter_context(tc.tile_pool(name="qt", bufs=1))
        sxp = ctx.enter_context(tc.tile_pool(name="sxp", bufs=1))
        stat = ctx.enter_context(tc.tile_pool(name="stat", bufs=3))
        ztp = ctx.enter_context(tc.tile_pool(name="ztp", bufs=6))
        znp = ctx.enter_context(tc.tile_pool(name="znp", bufs=2))
        yp = ctx.enter_context(tc.tile_pool(name="yp", bufs=2))
        psum = ctx.enter_context(tc.tile_pool(name="psum", bufs=6, space="PSUM"))
        tpp = ctx.enter_context(tc.tile_pool(name="tpp", bufs=2, space="PSUM"))

        # one contiguous [O, T] staging buffer: a token tile's transpose-back
        # is a single [O, 128] -> [128, O] xbar read
        zt_dram = dram.tile([O, T], BF16, tag="zt_dram", name="zt_dram")
        q_dram_h1 = dram.tile([TH, K], BF16, tag="q_dram1", name="q_dram1")
        rinv_dram = dram.tile([TT, P], F32, tag="rinv_dram")

        identity = singles.tile([P, P], BF16, tag="identity")
        make_identity(nc, identity)

        # bias row in bf16 (partition 0), for the K=1 bias matmul
        b_row = singles.tile([1, O], BF16, tag="b_row")
        nc.gpsimd.dma_start(out=b_row, in_=b_ext[:])  # gpsimd DMA casts f32->bf16

        # q^T strips, one per (token-half, k-tile): [128k, 512t] bf16
        qt_tiles = [
            [qt_pool.tile([P, TH], BF16, tag=f"qt{h}_{k}", name=f"qt{h}_{k}")
             for k in range(KT)]
            for h in range(2)
        ]

        KH = K // 2
        sx_tiles = [None] * TT
        rinv_rows = [None, None]

        def quant_tile(t, x_eng_of):
            """Load+quantize token tile t; build q^T (h0: PE, h1: DRAM)."""
            xh = []
            for i in range(2):
                x_half = stream.tile([P, KH], F32, tag="stream", name="x_half")
                x_eng_of(i).dma_start(
                    out=x_half, in_=x_ext[ds(t * P, P), ds(i * KH, KH)]
                )
                xh.append(x_half)
            amh = stat.tile([P, 2], F32, tag="am_x")
            for i in range(2):
                nc.vector.tensor_reduce(
                    out=amh[:, i:i + 1], in_=xh[i], axis=mybir.AxisListType.X,
                    op=mybir.AluOpType.max, apply_absolute_value=True,
                )
            am = stat.tile([P, 1], F32, tag="am_c")
            nc.vector.tensor_reduce(
                out=am, in_=amh, axis=mybir.AxisListType.X,
                op=mybir.AluOpType.max,
            )
            sx = sxp.tile([P, 1], F32, tag=f"sx{t}", name=f"sx{t}")
            # s = max(absmax, EPS) * (1/127)
            nc.vector.tensor_scalar(
                out=sx, in0=am, scalar1=EPS, scalar2=INV_QMAX,
                op0=mybir.AluOpType.max, op1=mybir.AluOpType.mult,
            )
            rinv = stat.tile([P, 1], F32, tag="rinv_x")
            nc.vector.reciprocal(out=rinv, in_=sx)
            nc.gpsimd.dma_start(out=rinv_dram[t, :], in_=rinv[:, 0:1])
            h, row = t // (TT // 2), (t % (TT // 2)) * P
            for i in range(2):
                # r = x * rinv + MAGIC  (in place, gpsimd), q = r - MAGIC -> bf16
                nc.gpsimd.tensor_scalar(
                    out=xh[i], in0=xh[i], scalar1=rinv, scalar2=MAGIC,
                    op0=mybir.AluOpType.mult, op1=mybir.AluOpType.add,
                )
                q_half = qp.tile([P, KH], BF16, tag="q_half")
                nc.vector.tensor_scalar(
                    out=q_half, in0=xh[i], scalar1=MAGIC,
                    scalar2=None, op0=mybir.AluOpType.subtract,
                )
                if h == 0:
                    # first token half: PE-transpose q into the q^T strips
                    # (PE is idle during the head)
                    for j in range(KT // 2):
                        k = i * (KT // 2) + j
                        tp = tpp.tile([P, P], BF16, tag="tp")
                        nc.tensor.transpose(
                            tp, q_half[:, ds(j * P, P)], identity
                        )
                        nc.scalar.copy(
                            out=qt_tiles[0][k][:, ds(row, P)], in_=tp
                        )
                else:
                    # second half: DRAM-staged xbar transposes (no hurry)
                    nc.gpsimd.dma_start(
                        out=q_dram_h1[ds(row, P), ds(i * KH, KH)], in_=q_half
                    )
            sx_tiles[t] = sx

        def load_rinv_row(h):
            # rinv as a bf16 row [1, TH] (rhs of the K=1 bias matmul),
            # per half so th=0's bias matmul never waits on t4-7's quant
            r = singles.tile([1, TH], BF16, tag=f"rinv_row{h}",
                             name=f"rinv_row{h}")
            nc.gpsimd.dma_start(out=r, in_=rinv_dram[ds(h * (TT // 2), TT // 2), :])
            rinv_rows[h] = r

        KB = 8                       # k-subtiles per W block
        NKB = KT // KB               # 4 blocks per o-group

        def matmul_og(th, og, w_engine_of, evac_eng):
            ps = [
                psum.tile([P, TH], F32, tag="ps", name=f"ps_{th}_{og}_{i}")
                for i in range(OT_PER_G)
            ]
            for kb in range(NKB):
                w_tile = stream.tile([P, KB, OG], BF16, tag="stream",
                                     name="w_tile")
                w_engine_of(kb).dma_start(
                    out=w_tile,
                    in_=wt_ext[
                        ds(kb * KB * P, KB * P), ds(og * OG, OG)
                    ].rearrange("(s p) o -> p s o", p=P),
                )
                for s in range(KB):
                    k = kb * KB + s
                    for ot in range(OT_PER_G):
                        nc.tensor.matmul(
                            ps[ot],
                            w_tile[:, s, ds(ot * P, P)],
                            qt_tiles[th][k],
                            start=(k == 0),
                            stop=False,
                        )
            # bias: psum += b_chunk^T @ rinv_row   (K=1 matmul)
            for ot in range(OT_PER_G):
                o0 = og * OG + ot * P
                nc.tensor.matmul(
                    ps[ot],
                    b_row[0:1, ds(o0, P)],
                    rinv_rows[th][0:1, :],
                    start=False,
                    stop=True,
                )
            for ot in range(OT_PER_G):
                zt_sb = ztp.tile([P, TH], BF16, tag="zt_sb")
                if evac_eng is nc.vector:
                    evac_eng.tensor_copy(out=zt_sb, in_=ps[ot])
                else:
                    evac_eng.copy(out=zt_sb, in_=ps[ot])
                # evacs are data-dependent: on a FIFO HWDGE ring they
                # head-of-line block later W prefetch, so use SWDGE
                nc.gpsimd.dma_start(
                    out=zt_dram[ds(og * OG + ot * P, P), ds(th * TH, TH)],
                    in_=zt_sb,
                )

        def pass2_tile(t, store_eng):
            """Transpose token tile t back to natural layout, requant, store."""
            z_nat = znp.tile([P, O], BF16, tag="z_nat")
            # single 1 MiB xbar transposed read: [O, 128] -> [128, O]
            nc.scalar.dma_start_transpose(
                z_nat, zt_dram[:, ds(t * P, P)]
            )
            # per-token absmax of y comes from z: absmax(y) = s_x * absmax(z)
            OH = O // 2
            am = stat.tile([P, 1], F32, tag="am_z")
            nc.vector.tensor_reduce(
                out=am, in_=z_nat, axis=mybir.AxisListType.X,
                op=mybir.AluOpType.max, apply_absolute_value=True,
            )
            sy = stat.tile([P, 1], F32, tag="sy")
            # sy = (max(am * sx, EPS)) * (1/127)
            nc.vector.tensor_scalar(
                out=sy, in0=am, scalar1=sx_tiles[t], scalar2=EPS,
                op0=mybir.AluOpType.mult, op1=mybir.AluOpType.max,
            )
            nc.vector.tensor_scalar(
                out=sy, in0=sy, scalar1=INV_QMAX, scalar2=None,
                op0=mybir.AluOpType.mult,
            )
            rinvy = stat.tile([P, 1], F32, tag="rinv_y")
            nc.vector.reciprocal(out=rinvy, in_=sy)
            # f1 = s_x * rinv_y: ONE scalar-engine activation then does
            # r = z*f1 + MAGIC (bias already inside z)
            f1 = stat.tile([P, 1], F32, tag="f1")
            nc.vector.tensor_scalar(
                out=f1, in0=rinvy, scalar1=sx_tiles[t], scalar2=None,
                op0=mybir.AluOpType.mult,
            )
            for i in range(2):
                y_half = yp.tile([P, OH], F32, tag="y_half")
                # r = z * (sx*rinvy) + MAGIC  (scalar ACT, fused affine)
                nc.scalar.activation(
                    out=y_half, in_=z_nat[:, ds(i * OH, OH)],
                    func=mybir.ActivationFunctionType.Copy,
                    bias=MAGIC, scale=f1,
                )
                # y_q = (r - MAGIC) * s_y  (in place; vector ONLY - gpsimd
                # tensor_scalar with an AP in the scalar2 slot measured
                # 29.3us/op vs 1.1us on vector)
                nc.vector.tensor_scalar(
                    out=y_half, in0=y_half, scalar1=MAGIC, scalar2=sy,
                    op0=mybir.AluOpType.subtract, op1=mybir.AluOpType.mult,
                )
                store_eng.dma_start(
                    out=out_ext[ds(t * P, P), ds(i * OH, OH)], in_=y_half
                )

        # ---- quant tiles 0-3 (x-h0 split across both rings) ----
        for t in range(4):
            quant_tile(t, x_eng_of=lambda i: nc.sync if i == 0 else nc.scalar)
        load_rinv_row(0)

        # ---- matmul mega-pass th=0 ----
        # W rings: og0-2 all-scalar (sync drains x-h0), og3-5 split,
        # og6-7 all-sync (scalar drains the qt-h1 transpose batch)
        for og in range(NOG):
            if og < 3:
                w_engine_of = lambda kb: nc.scalar
            elif og < 6:
                w_engine_of = lambda kb: nc.sync if kb % 2 == 0 else nc.scalar
            else:
                w_engine_of = lambda kb: nc.sync
            matmul_og(0, og, w_engine_of=w_engine_of, evac_eng=nc.scalar)
            if og == 3:
                # x-h1 + quant for tiles 4-7, in og0-3's shadow (slots
                # freed at quant speed, not matmul speed)
                for t in range(4, TT):
                    quant_tile(t, x_eng_of=lambda i: nc.sync)
                load_rinv_row(1)
            if og == 5:
                # all 32 qt-h1 strips; q_dram_h1 is complete by now
                for k in range(KT):
                    nc.scalar.dma_start_transpose(
                        qt_tiles[1][k], q_dram_h1[:, ds(k * P, P)]
                    )

        # ---- matmul mega-pass th=1: W entirely on the sync ring; evacs on
        # VectorE (ScalarE carries pass-2A's ACTs + transposes then).
        # pass-2A (tokens 0-511) is emitted after og1 so its DMA-lane
        # completions can't stall og0/og1's W waits ----
        for og in range(NOG):
            matmul_og(
                1, og,
                w_engine_of=lambda kb: nc.sync,
                evac_eng=nc.vector,
            )
            if og == 1:
                for t in range(TT // 2):
                    pass2_tile(t, store_eng=nc.scalar)

        # ---- pass-2B tail: token tiles 4..7 ----
        for t in range(TT // 2, TT):
            pass2_tile(t, store_eng=nc.sync)

    nc.compile()
    return nc


_NC_CACHE = None


def _get_nc():
    global _NC_CACHE
    if _NC_CACHE is None:
        _NC_CACHE = build()
    return _NC_CACHE


def _run(x, W, b, trace=False):
    nc = _get_nc()
    x2d = np.ascontiguousarray(np.asarray(x, dtype=np.float32).reshape(-1, K))
    wt = np.ascontiguousarray(np.asarray(W, dtype=np.float32).T).astype(
        ml_dtypes.bfloat16
    )
    bf = np.ascontiguousarray(np.asarray(b, dtype=np.float32))
    in_maps = [
        {"x": np.ascontiguousarray(x2d[i * T:(i + 1) * T]), "wt": wt, "b": bf}
        for i in range(N_CORES)
    ]
    res = run_bass_kernel_spmd(nc, in_maps, list(range(N_CORES)), trace=trace)
    out = np.concatenate([res.results[i]["out"] for i in range(N_CORES)], axis=0)
    return out, res


def kernel(x, W, b):
    out, _ = _run(x, W, b, trace=False)
    return out.reshape(np.asarray(x).shape[:-1] + (O,)).astype(np.float32)


# revision 17
# speedup vs baseline: 1.0802x; 1.0369x over previous
"""Trainium2 Bass kernel for a per-token fake-quantized Linear:

    y = fake_quant(fake_quant(x) @ W.T + b)      (per-token int8 symmetric)

x: [4, 2048, 4096] f32, W: [4096, 4096] f32, b: [4096] f32.

Strategy (8 NeuronCores, pure data parallel over tokens - zero collectives):
  - 8192 tokens / 8 cores = 1024 tokens per core; W, b replicated.
  - Per-token quantized x values are integers in [-127, 127], EXACTLY
    representable in bf16, so the matmul runs on TensorE in bf16 (integer
    q as the moving operand, host-pre-packed W.T bf16 stationary) with f32
    PSUM accumulation. The only precision loss vs the f32 reference is W's
    bf16 rounding (~0.8% rel err after output requant; gate is 2e-2).
  - Rounding is exact round-to-nearest-even via +/-1.5*2^23 magic adds.
  - The bias is folded into the matmul as a K=1 rank-1 update
    (b_chunk^T @ rinv_row_h) since s_x * rinv_x == 1.
  - z^T = Wb @ q^T is computed in output-transposed layout (W stationary,
    read once per token-half mega-pass); per-token output stats use
    absmax(y) = s_x * absmax(z) after an xbar transpose back to natural
    layout via one contiguous [O, T] DRAM staging buffer (one 1 MiB
    transposed read per token tile).
  - Scheduling (the perf-critical part; each point trace-verified):
      * x halves and W blocks SHARE one 10-slot SBUF pool (equal 8 KiB
        per-partition slots). A separate W pool paces W-issue off matmul
        completion via slot-WAR semaphores and the pipeline never fills
        (8-35us PE gaps at og0-3); sharing lets early W reuse x slots
        freed at quant speed.
      * Quant is emitted in two batches: tiles 0-3 before the og loop,
        tiles 4-7 between og3 and og4. x-h1 then loads in og0-3's shadow
        and og0-3's zt evac-writes are not queued behind t4-7's
        quant work on gpsimd.
      * rinv_row is per-half so og0's bias matmul never waits on t4-7.
      * th=0 evacs on ScalarE, th=1 on VectorE (whichever queue is free).
      * qt-h1 xbar strips: one batch after og5 (q_dram ready ~115us),
        og6-7's W moves fully to sync so the batch can't delay it.
      * pass-2 for tokens 0-511 is emitted after th=1's og1 so its
        DMA-lane completions can't stall th1-og0/og1's W waits; it
        executes under th1's matmuls. Only tokens 512-1023's pass-2
        remains as the tail.
      * All xbar transposes stay on the scalar ring (transpose on scalar
        concurrent with copy on sync is the proven-safe combo).
"""

import sys

if "/opt/trn_rl_repo" not in sys.path:
    sys.path.insert(0, "/opt/trn_rl_repo")

from contextlib import ExitStack

import ml_dtypes
import numpy as np

import concourse.bass as bass
import concourse.mybir as mybir
import concourse.tile as tile
from concourse import bacc
from concourse.bass import ds
from concourse.bass_utils import run_bass_kernel_spmd
from concourse.masks import make_identity

N_CORES = 8
P = 128
T = 1024          # tokens per core
K = 4096          # in features
O = 4096          # out features
TT = T // P       # 8 token tiles
KT = K // P       # 32 k tiles
TH = T // 2       # token half (512) = matmul N
OG = 512          # outputs per o-group (4 o-tiles -> 4 PSUM banks in flight)
NOG = O // OG     # 8 o-groups
OT_PER_G = OG // P  # 4

Q_MAX = 127.0
EPS = 1e-5
MAGIC = 1.5 * 2**23  # f32 add/sub forces round-to-nearest-even to integer
INV_QMAX = float(np.float32(1.0) / np.float32(Q_MAX))

F32 = mybir.dt.float32
BF16 = mybir.dt.bfloat16


def build():
    nc = bacc.Bacc()
    x_ext = nc.declare_dram_parameter("x", [T, K], F32, isOutput=False)
    wt_ext = nc.declare_dram_parameter("wt", [K, O], BF16, isOutput=False)
    b_ext = nc.declare_dram_parameter("b", [O], F32, isOutput=False)
    out_ext = nc.declare_dram_parameter("out", [T, O], F32, isOutput=True)

    with tile.TileContext(nc) as tc, ExitStack() as ctx:
        dram = ctx.enter_context(tc.tile_pool(name="dram", bufs=1, space="DRAM"))
        singles = ctx.enter_context(tc.tile_pool(name="singles", bufs=1))
        # x halves AND W blocks share this pool: both are 8 KiB/partition
        stream = ctx.enter_context(tc.tile_pool(name="stream", bufs=10))
        qp = ctx.enter_context(tc.tile_pool(name="qp", bufs=3))
        qt_pool = ctx.enter_context(tc.tile_pool(name="qt", bufs=1))
        sxp = ctx.enter_context(tc.tile_pool(name="sxp", bufs=1))
        stat = ctx.enter_context(tc.tile_pool(name="stat", bufs=3))
        ztp = ctx.enter_context(tc.tile_pool(name="ztp", bufs=6))
        znp = ctx.enter_context(tc.tile_pool(name="znp", bufs=2))
        yp = ctx.enter_context(tc.tile_pool(name="yp", bufs=2))
        psum = ctx.enter_context(tc.tile_pool(name="psum", bufs=6, space="PSUM"))
        tpp = ctx.enter_context(tc.tile_pool(name="tpp", bufs=2, space="PSUM"))

        # one contiguous [O, T] staging buffer: a token tile's transpose-back
        # is a single [O, 128] -> [128, O] xbar read
        zt_dram = dram.tile([O, T], BF16, tag="zt_dram", name="zt_dram")
        q_dram_h1 = dram.tile([TH, K], BF16, tag="q_dram1", name="q_dram1")
        rinv_dram = dram.tile([TT, P], F32, tag="rinv_dram")

        identity = singles.tile([P, P], BF16, tag="identity")
        make_identity(nc, identity)

        # bias row in bf16 (partition 0), for the K=1 bias matmul
        b_row = singles.tile([1, O], BF16, tag="b_row")
        nc.gpsimd.dma_start(out=b_row, in_=b_ext[:])  # gpsimd DMA casts f32->bf16

        # q^T strips, one per (token-half, k-tile): [128k, 512t] bf16
        qt_tiles = [
            [qt_pool.tile([P, TH], BF16, tag=f"qt{h}_{k}", name=f"qt{h}_{k}")
             for k in range(KT)]
            for h in range(2)
        ]

        KH = K // 2
        sx_tiles = [None] * TT
        rinv_rows = [None, None]

        def quant_tile(t, x_eng_of):
            """Load+quantize token tile t; build q^T (h0: PE, h1: DRAM)."""
            xh = []
            for i in range(2):
                x_half = stream.tile([P, KH], F32, tag="stream", name="x_half")
                x_eng_of(i).dma_start(
                    out=x_half, in_=x_ext[ds(t * P, P), ds(i * KH, KH)]
                )
                xh.append(x_half)
            amh = stat.tile([P, 2], F32, tag="am_x")
            for i in range(2):
                nc.vector.tensor_reduce(
                    out=amh[:, i:i + 1], in_=xh[i], axis=mybir.AxisListType.X,
                    op=mybir.AluOpType.max, apply_absolute_value=True,
                )
            am = stat.tile([P, 1], F32, tag="am_c")
            nc.vector.tensor_reduce(
                out=am, in_=amh, axis=mybir.AxisListType.X,
                op=mybir.AluOpType.max,
            )
            sx = sxp.tile([P, 1], F32, tag=f"sx{t}", name=f"sx{t}")
            # s = max(absmax, EPS) * (1/127)
            nc.vector.tensor_scalar(
                out=sx, in0=am, scalar1=EPS, scalar2=INV_QMAX,
                op0=mybir.AluOpType.max, op1=mybir.AluOpType.mult,
            )
            rinv = stat.tile([P, 1], F32, tag="rinv_x")
            nc.vector.reciprocal(out=rinv, in_=sx)
            nc.gpsimd.dma_start(out=rinv_dram[t, :], in_=rinv[:, 0:1])
            h, row = t // (TT // 2), (t % (TT // 2)) * P
            for i in range(2):
                # r = x * rinv + MAGIC  (in place, gpsimd), q = r - MAGIC -> bf16
                nc.gpsimd.tensor_scalar(
                    out=xh[i], in0=xh[i], scalar1=rinv, scalar2=MAGIC,
                    op0=mybir.AluOpType.mult, op1=mybir.AluOpType.add,
                )
                q_half = qp.tile([P, KH], BF16, tag="q_half")
                nc.vector.tensor_scalar(
                    out=q_half, in0=xh[i], scalar1=MAGIC,
                    scalar2=None, op0=mybir.AluOpType.subtract,
                )
                if h == 0:
                    # first token half: PE-transpose q into the q^T strips
                    # (PE is idle during the head)
                    for j in range(KT // 2):
                        k = i * (KT // 2) + j
                        tp = tpp.tile([P, P], BF16, tag="tp")
                        nc.tensor.transpose(
                            tp, q_half[:, ds(j * P, P)], identity
                        )
                        nc.scalar.copy(
                            out=qt_tiles[0][k][:, ds(row, P)], in_=tp
                        )
                else:
                    # second half: DRAM-staged xbar transposes (no hurry)
                    nc.gpsimd.dma_start(
                        out=q_dram_h1[ds(row, P), ds(i * KH, KH)], in_=q_half
                    )
            sx_tiles[t] = sx

        def load_rinv_row(h):
            # rinv as a bf16 row [1, TH] (rhs of the K=1 bias matmul),
            # per half so th=0's bias matmul never waits on t4-7's quant
            r = singles.tile([1, TH], BF16, tag=f"rinv_row{h}",
                             name=f"rinv_row{h}")
            nc.gpsimd.dma_start(out=r, in_=rinv_dram[ds(h * (TT // 2), TT // 2), :])
            rinv_rows[h] = r

        KB = 8                       # k-subtiles per W block
        NKB = KT // KB               # 4 blocks per o-group

        def matmul_og(th, og, w_engine_of, evac_eng):
            ps = [
                psum.tile([P, TH], F32, tag="ps", name=f"ps_{th}_{og}_{i}")
                for i in range(OT_PER_G)
            ]
            for kb in range(NKB):
                w_tile = stream.tile([P, KB, OG], BF16, tag="stream",
                                     name="w_tile")
                w_engine_of(kb).dma_start(
                    out=w_tile,
                    in_=wt_ext[
                        ds(kb * KB * P, KB * P), ds(og * OG, OG)
                    ].rearrange("(s p) o -> p s o", p=P),
                )
                for s in range(KB):
                    k = kb * KB + s
                    for ot in range(OT_PER_G):
                        nc.tensor.matmul(
                            ps[ot],
                            w_tile[:, s, ds(ot * P, P)],
                            qt_tiles[th][k],
                            start=(k == 0),
                            stop=False,
                        )
            # bias: psum += b_chunk^T @ rinv_row   (K=1 matmul)
            for ot in range(OT_PER_G):
                o0 = og * OG + ot * P
                nc.tensor.matmul(
                    ps[ot],
                    b_row[0:1, ds(o0, P)],
                    rinv_rows[th][0:1, :],
                    start=False,
                    stop=True,
                )
            for ot in range(OT_PER_G):
                zt_sb = ztp.tile([P, TH], BF16, tag="zt_sb")
                if evac_eng is nc.vector:
                    evac_eng.tensor_copy(out=zt_sb, in_=ps[ot])
                else:
                    evac_eng.copy(out=zt_sb, in_=ps[ot])
                # evacs are data-dependent: on a FIFO HWDGE ring they
                # head-of-line block later W prefetch, so use SWDGE
                nc.gpsimd.dma_start(
                    out=zt_dram[ds(og * OG + ot * P, P), ds(th * TH, TH)],
                    in_=zt_sb,
                )

        def pass2_tile(t, store_eng):
            """Transpose token tile t back to natural layout, requant, store."""
            z_nat = znp.tile([P, O], BF16, tag="z_nat")
            # single 1 MiB xbar transposed read: [O, 128] -> [128, O]
            nc.scalar.dma_start_transpose(
                z_nat, zt_dram[:, ds(t * P, P)]
            )
            # per-token absmax of y comes from z: absmax(y) = s_x * absmax(z)
            OH = O // 2
            am = stat.tile([P, 1], F32, tag="am_z")
            nc.vector.tensor_reduce(
                out=am, in_=z_nat, axis=mybir.AxisListType.X,
                op=mybir.AluOpType.max, apply_absolute_value=True,
            )
            sy = stat.tile([P, 1], F32, tag="sy")
            # sy = (max(am * sx, EPS)) * (1/127)
            nc.vector.tensor_scalar(
                out=sy, in0=am, scalar1=sx_tiles[t], scalar2=EPS,
                op0=mybir.AluOpType.mult, op1=mybir.AluOpType.max,
            )
            nc.vector.tensor_scalar(
                out=sy, in0=sy, scalar1=INV_QMAX, scalar2=None,
                op0=mybir.AluOpType.mult,
            )
            rinvy = stat.tile([P, 1], F32, tag="rinv_y")
            nc.vector.reciprocal(out=rinvy, in_=sy)
            # f1 = s_x * rinv_y: ONE scalar-engine activation then does
            # r = z*f1 + MAGIC (bias already inside z)
            f1 = stat.tile([P, 1], F32, tag="f1")
            nc.vector.tensor_scalar(
                out=f1, in0=rinvy, scalar1=sx_tiles[t], scalar2=None,
                op0=mybir.AluOpType.mult,
            )
            for i in range(2):
                y_half = yp.tile([P, OH], F32, tag="y_half")
                # r = z * (sx*rinvy) + MAGIC  (scalar ACT, fused affine)
                nc.scalar.activation(
                    out=y_half, in_=z_nat[:, ds(i * OH, OH)],
                    func=mybir.ActivationFunctionType.Copy,
                    bias=MAGIC, scale=f1,
                )
                # y_q = (r - MAGIC) * s_y  (in place; vector ONLY - gpsimd
                # tensor_scalar with an AP in the scalar2 slot measured
                # 29.3us/op vs 1.1us on vector)
                nc.vector.tensor_scalar(
                    out=y_half, in0=y_half, scalar1=MAGIC, scalar2=sy,
                    op0=mybir.AluOpType.subtract, op1=mybir.AluOpType.mult,
                )
                store_eng.dma_start(
                    out=out_ext[ds(t * P, P), ds(i * OH, OH)], in_=y_half
                )

        # ---- quant tiles 0-3 (x-h0 split across both rings) ----
        for t in range(4):
            quant_tile(t, x_eng_of=lambda i: nc.sync if i == 0 else nc.scalar)
        load_rinv_row(0)

        # ---- matmul mega-pass th=0 ----
        # W rings: og0 behind x-i0 on sync, og1-2 behind x-i1 on scalar
        # (each ring's FIFO gives x strict priority over W - issuing W on
        # the ring x is NOT on stole half of x's HBM bandwidth and pushed
        # the first matmul from ~33us to ~53us), og3-5 split, og6-7 sync
        # (scalar drains the qt-h1 transpose batch then).
        # Evacs: og0-4 on ScalarE; og5-7 on VectorE - og6/7's evacs on
        # scalar would queue behind the 32 qt transposes (~40us) and
        # stall og7 on psum slots (measured 60.7us).
        for og in range(NOG):
            if og == 0:
                w_engine_of = lambda kb: nc.sync
            elif og < 3:
                w_engine_of = lambda kb: nc.scalar
            elif og < 6:
                w_engine_of = lambda kb: nc.sync if kb % 2 == 0 else nc.scalar
            else:
                w_engine_of = lambda kb: nc.sync
            matmul_og(0, og, w_engine_of=w_engine_of,
                      evac_eng=nc.scalar if og < 5 else nc.vector)
            if og == 3:
                # x-h1 + quant for tiles 4-7, in og0-3's shadow (slots
                # freed at quant speed, not matmul speed)
                for t in range(4, TT):
                    quant_tile(t, x_eng_of=lambda i: nc.sync)
                load_rinv_row(1)
            if og == 5:
                # all 32 qt-h1 strips; q_dram_h1 is complete by now
                for k in range(KT):
                    nc.scalar.dma_start_transpose(
                        qt_tiles[1][k], q_dram_h1[:, ds(k * P, P)]
                    )

        # ---- matmul mega-pass th=1: W entirely on the sync ring; evacs on
        # VectorE (ScalarE carries pass-2A's ACTs + transposes then).
        # pass-2A (tokens 0-511) is interleaved ONE TILE PER OG starting
        # after og1: a single block would put ~25us of pass-2A vector work
        # ahead of og2+'s evacs in the queue and stall og4+ on psum slots;
        # starting before og1 would stall og0/og1's W waits on shared
        # DMA-completion lanes behind pass-2A's transposes ----
        for og in range(NOG):
            matmul_og(
                1, og,
                w_engine_of=lambda kb: nc.sync,
                evac_eng=nc.vector,
            )
            if 1 <= og <= 4:
                pass2_tile(og - 1, store_eng=nc.scalar)

        # ---- pass-2B tail: token tiles 4..7 ----
        for t in range(TT // 2, TT):
            pass2_tile(t, store_eng=nc.sync)

    nc.compile()
    return nc


_NC_CACHE = None


def _get_nc():
    global _NC_CACHE
    if _NC_CACHE is None:
        _NC_CACHE = build()
    return _NC_CACHE


def _run(x, W, b, trace=False):
    nc = _get_nc()
    x2d = np.ascontiguousarray(np.asarray(x, dtype=np.float32).reshape(-1, K))
    wt = np.ascontiguousarray(np.asarray(W, dtype=np.float32).T).astype(
        ml_dtypes.bfloat16
    )
    bf = np.ascontiguousarray(np.asarray(b, dtype=np.float32))
    in_maps = [
        {"x": np.ascontiguousarray(x2d[i * T:(i + 1) * T]), "wt": wt, "b": bf}
        for i in range(N_CORES)
    ]
    res = run_bass_kernel_spmd(nc, in_maps, list(range(N_CORES)), trace=trace)
    out = np.concatenate([res.results[i]["out"] for i in range(N_CORES)], axis=0)
    return out, res


def kernel(x, W, b):
    out, _ = _run(x, W, b, trace=False)
    return out.reshape(np.asarray(x).shape[:-1] + (O,)).astype(np.float32)
